# revision 1
# baseline (speedup 1.0000x reference)
"""Trainium2 Bass kernel v5: f16x2 spike layers, fully chunk-gated GEMM.

The v2/v4 configurations are DMA-paced (~119MB of weights+IO per core per
invocation). v5 cuts spike-layer weights to 4B/weight: hi = rne11(W) in
f16 (exact 11-bit), lo = (W - hi)*2^11 in f16 (11 more bits) -> 22-bit
weights, 2 matmuls, DMA 16.8MB/layer (was 25.2). Layer 0 keeps the safe
26-bit f32r x/W 3-term split (x quantization is the most flip-sensitive).

Being all-f16, every spike-layer matmul is chunkable (no f32r >=256-row
rule): per (f-chunk c, mo) one own-bank PSUM mini-group of 32 matmuls
(16 ko hi + 16 ko lo), gated on the previous layer's scan reaching chunk
c. The PE fills the scan window with real GEMM work; no hi/add phase.

Spikes: s_hi16 (f16, 0/1) extracted on DVE per chunk; s_lo16 = s_hi16 *
2^-11 (f16, exact) derived on Act.
"""

import os
import sys

sys.path.insert(0, "/opt/trn_rl_repo")

import numpy as np

B, T, N = 32, 100, 2048
NL = 4
NCORES = 8
BL = B // NCORES          # 4
NO = N // 128             # 16
KO = N // 128             # 16
F = T * BL                # 400
PSF = 512                 # one full 2KB PSUM bank
ALPHA = float(np.float32(np.exp(np.float32(-1.0 / 20.0))))
HI_BITS = int(os.environ.get("LIF_HI_BITS", "12"))      # layer-0 f32r split
F16_BITS = 11                                           # f16 significand
SLO16 = float(2.0 ** -F16_BITS)
NCH = int(os.environ.get("LIF_NCH", "4"))
TCH = T // NCH
FCH = TCH * BL
GRP = 8


def build(reps: int = 1, internal_weights: bool = False):
    import contextlib
    from concourse import mybir, bacc
    import concourse.tile as tile

    dt = mybir.dt
    nc = bacc.Bacc("TRN2", target_bir_lowering=False, debug=False,
                   num_devices=NCORES)
    wkind = {} if internal_weights else {"kind": "ExternalInput"}
    # layer-0 f32r hi + lo weights
    wh0_d = nc.dram_tensor("wh0", [NO, 128, KO, 128], dt.float32r,
                           **wkind).ap()
    wl0_d = nc.dram_tensor("wl0", [NO, 128, KO, 128], dt.float32r,
                           **wkind).ap()
    # spike layers: packed f16 hi/lo pairs
    wf_d = nc.dram_tensor("wf", [NL - 1, NO, 128, KO, 2, 128], dt.float16,
                          **wkind).ap()
    xh_d = nc.dram_tensor("xh", [128, KO, F], dt.float32r,
                          kind="ExternalInput").ap()
    xl_d = nc.dram_tensor("xl", [128, KO, F], dt.float32r,
                          kind="ExternalInput").ap()
    out_d = nc.dram_tensor("out", [128, NO, F], dt.float32,
                           kind="ExternalOutput").ap()

    with tile.TileContext(nc) as tctx:
        with contextlib.ExitStack() as stack:
            actsp = stack.enter_context(tctx.tile_pool(name="acts", bufs=2))
            acts16p = stack.enter_context(tctx.tile_pool(name="acts16",
                                                         bufs=2))
            whp = stack.enter_context(tctx.tile_pool(name="whp", bufs=2))
            wl0p = stack.enter_context(tctx.tile_pool(name="wl0p", bufs=2))
            wfp = stack.enter_context(tctx.tile_pool(name="wfp", bufs=GRP))
            curp = stack.enter_context(tctx.tile_pool(name="curp", bufs=1))
            vp = stack.enter_context(tctx.tile_pool(name="vp", bufs=2))
            pp = stack.enter_context(tctx.tile_pool(name="pp", bufs=1,
                                                    space="PSUM"))

            def psum_tile(name, mo):
                return pp.tile([128, PSF], dt.float32, tag=f"pt{mo % 8}",
                               name=name)

            def body(_iv=None):
                xh = actsp.tile([128, KO, F], dt.float32r, tag="acts",
                                name="xh_t")
                xl = actsp.tile([128, KO, F], dt.float32r, tag="acts",
                                name="xl_t")
                for kg in range(4):
                    ksl = slice(kg * 4, (kg + 1) * 4)
                    nc.sync.dma_start(xh[:, ksl, :], xh_d[:, ksl, :])
                    nc.sync.dma_start(xl[:, ksl, :], xl_d[:, ksl, :])

                s_hi = xh      # f16 {0,1} spikes for l>=1; xh for l=0
                s_lo = xl      # f16 {0,2^-11} spikes for l>=1; xl for l=0

                for l in range(NL):
                    cur = curp.tile([128, NO, F], dt.float32, tag="cur",
                                    name=f"cur_{l}")
                    if l == 0:
                        for mo in range(NO):
                            wh = whp.tile([128, KO, 128], dt.float32r,
                                          tag="wh", name=f"wh0_{mo}")
                            nc.sync.dma_start(wh[:, :, :], wh0_d[mo])
                            wl = wl0p.tile([128, KO, 128], dt.float32r,
                                           tag="wl0", name=f"wl0_{mo}")
                            nc.sync.dma_start(wl[:, :, :], wl0_d[mo])
                            pt = psum_tile(f"pt0_{mo}", mo)
                            for ko in range(KO):
                                nc.tensor.matmul(pt[:, :F], wh[:, ko, :],
                                                 s_hi[:, ko, :],
                                                 start=(ko == 0), stop=False)
                            for ko in range(KO):
                                nc.tensor.matmul(pt[:, :F], wh[:, ko, :],
                                                 s_lo[:, ko, :],
                                                 start=False, stop=False)
                            for ko in range(KO):
                                nc.tensor.matmul(pt[:, :F], wl[:, ko, :],
                                                 s_hi[:, ko, :],
                                                 start=False,
                                                 stop=(ko == KO - 1))
                            nc.scalar.copy(cur[:, mo, :], pt[:, :F])
                    else:
                        for g0 in range(0, NO, GRP):
                            wts = {}
                            for mo in range(g0, g0 + GRP):
                                wt = wfp.tile([128, KO, 2, 128], dt.float16,
                                              tag="wf", name=f"wf_{l}_{mo}")
                                nc.sync.dma_start(wt[:, :, :, :],
                                                  wf_d[l - 1, mo])
                                wts[mo] = wt
                            for c in range(NCH):
                                csl = slice(c * FCH, (c + 1) * FCH)
                                for mo in range(g0, g0 + GRP):
                                    wt = wts[mo]
                                    pt = psum_tile(f"pt_{l}_{c}_{mo}", mo)
                                    for ko in range(KO):
                                        nc.tensor.matmul(
                                            pt[:, :FCH], wt[:, ko, 0, :],
                                            s_hi[:, ko, csl],
                                            start=(ko == 0), stop=False)
                                    for ko in range(KO):
                                        nc.tensor.matmul(
                                            pt[:, :FCH], wt[:, ko, 1, :],
                                            s_lo[:, ko, csl],
                                            start=False,
                                            stop=(ko == KO - 1))
                                    nc.scalar.copy(cur[:, mo, csl],
                                                   pt[:, :FCH])

                    # --- LIF scan + chunked extraction ---
                    vt = vp.tile([128, NO, BL], dt.float32, tag="v",
                                 name=f"v_{l}")
                    nc.vector.memset(vt[:, :, :], 0.0)
                    if l == NL - 1:
                        spk = actsp.tile([128, NO, F], dt.float32,
                                         tag="acts", name="spk_out")
                    else:
                        s_hi = actsp.tile([128, NO, F], dt.float16,
                                          tag="acts", name=f"shi_{l}")
                        s_lo = acts16p.tile([128, NO, F], dt.float16,
                                            tag="acts16", name=f"slo_{l}")
                    for t in range(T):
                        tl = slice(t * BL, (t + 1) * BL)
                        nc.vector.scalar_tensor_tensor(
                            cur[:, :, tl], vt[:, :, :], ALPHA, cur[:, :, tl],
                            op0=mybir.AluOpType.mult,
                            op1=mybir.AluOpType.add)
                        if t < T - 1:
                            nc.vector.scalar_tensor_tensor(
                                vt[:, :, :], cur[:, :, tl], 1.0,
                                cur[:, :, tl],
                                op0=mybir.AluOpType.is_lt,
                                op1=mybir.AluOpType.mult)
                        if (t + 1) % TCH == 0:
                            c = (t + 1) // TCH - 1
                            csl = slice(c * FCH, (c + 1) * FCH)
                            if l == NL - 1:
                                nc.vector.tensor_scalar(
                                    spk[:, :, csl], cur[:, :, csl], 1.0,
                                    None, op0=mybir.AluOpType.is_ge)
                                nc.sync.dma_start(out_d[:, :, csl],
                                                  spk[:, :, csl])
                            else:
                                nc.vector.tensor_scalar(
                                    s_hi[:, :, csl], cur[:, :, csl], 1.0,
                                    None, op0=mybir.AluOpType.is_ge)
                                nc.scalar.mul(s_lo[:, :, csl],
                                              s_hi[:, :, csl], SLO16)

            if reps == 1:
                body()
            else:
                with tctx.For_i(0, reps, 1) as iv:
                    body(iv)
    nc.compile()
    return nc


def _rne_bits(a, bits):
    a = np.asarray(a, np.float64)
    m, e = np.frexp(a)
    sc = np.float64(2.0) ** bits
    return np.ldexp(np.round(m * sc) / sc, e).astype(np.float32)


def _chunk(wm):
    wmT = np.ascontiguousarray(wm.T)
    return np.ascontiguousarray(
        wmT.reshape(KO, 128, NO, 128).transpose(2, 1, 0, 3))


def prep_weights(inputs):
    wh0 = np.empty((NO, 128, KO, 128), np.float32)
    wl0 = np.empty((NO, 128, KO, 128), np.float32)
    wf = np.empty((NL - 1, NO, 128, KO, 2, 128), np.float16)
    for l in range(NL):
        wm = (np.asarray(inputs[f"W{l}"], np.float32)
              * np.asarray(inputs[f"mask{l}"]).astype(np.float32))
        wc = _chunk(wm)
        if l == 0:
            h = _rne_bits(wc, HI_BITS)
            wh0[:] = h
            wl0[:] = _rne_bits(wc.astype(np.float64) - h.astype(np.float64),
                               HI_BITS)
        else:
            h = _rne_bits(wc, F16_BITS)
            lo = ((wc.astype(np.float64) - h.astype(np.float64))
                  * np.float64(2.0 ** F16_BITS)).astype(np.float16)
            wf[l - 1, :, :, :, 0, :] = h.astype(np.float16)
            wf[l - 1, :, :, :, 1, :] = lo
    return {"wh0": wh0, "wl0": wl0, "wf": wf}


def prep_x(x_core):
    xt = x_core.transpose(2, 1, 0)
    xt = xt.reshape(KO, 128, T, BL).transpose(1, 0, 2, 3)
    x = np.ascontiguousarray(xt.reshape(128, KO, F), dtype=np.float32)
    xh = _rne_bits(x, HI_BITS)
    xl = (x.astype(np.float64) - xh.astype(np.float64)).astype(np.float32)
    xl = _rne_bits(xl, HI_BITS)
    return xh, xl


def unprep_out(o):
    o = o.reshape(128, NO, T, BL).transpose(1, 0, 2, 3)
    o = o.reshape(N, T, BL).transpose(2, 1, 0)
    return np.ascontiguousarray(o)


_cached_nc = None


def make_in_maps(inputs):
    wmaps = prep_weights(inputs)
    x = np.asarray(inputs["x"], np.float32)
    in_maps = []
    for ci in range(NCORES):
        xh, xl = prep_x(x[ci * BL:(ci + 1) * BL])
        in_maps.append(dict(wmaps, xh=xh, xl=xl))
    return in_maps


def kernel(**inputs) -> np.ndarray:
    global _cached_nc
    from concourse.bass_utils import run_bass_kernel_spmd

    if _cached_nc is None:
        _cached_nc = build(reps=1)
    nc = _cached_nc

    in_maps = make_in_maps(inputs)
    res = run_bass_kernel_spmd(nc, in_maps, core_ids=list(range(NCORES)))
    out = np.empty((B, T, N), np.float32)
    for ci in range(NCORES):
        out[ci * BL:(ci + 1) * BL] = unprep_out(res.results[ci]["out"])
    return out

